# revision 23
# baseline (speedup 1.0000x reference)
"""DiffusionNet kernel for 8 trn2 NeuronCores (data-parallel over batch).

Per core (16 samples): the cross-attn kv projection (phase 0) is emitted
inside cross-attn layer 0's sample-group loop (group g only needs the
context rows of its own 4 samples), so its dense PE/DMA work overlaps
the latency-bound token-path chains and keeps the PE HAM-warm; the kv
staging pools are closed afterwards so the FF weight pool reuses their
SBUF.  Token-path transposes go through the PE (+copy) instead of DMA
round trips; SwiGLU uses tanh so every ACT function stays in the
exp_and_others table set.
"""
import sys

for p in ("/opt/trn_rl_repo", "/root/.axon_site/_ro/trn_rl_repo"):
    if p not in sys.path:
        sys.path.insert(0, p)

import math
from contextlib import ExitStack

import numpy as np

import concourse.bass as bass
import concourse.bacc as bacc
import concourse.tile as tile
from concourse import mybir
from concourse.bass_utils import run_bass_kernel_spmd

F16 = mybir.dt.float16
F32 = mybir.dt.float32
AF = mybir.ActivationFunctionType
ALU = mybir.AluOpType

B, M, D = 128, 512, 512
H, DH = 8, 64
L = 6
INNER = 2048
ROT = 32
NCORES = 8
BS = B // NCORES          # 16 samples per core
T = 3 * BS                # 48 token rows per core
R = BS * M                # 8192 context rows per core
NEG = -1e30
SWAP16 = [(p + 16) % 32 for p in range(32)]
IDM32 = list(range(32))


def dt_kscr_slice(kscr, p_, q_):
    a = kscr.ap()          # [3, R, 128]
    return bass.AP(tensor=a.tensor,
                   offset=a.offset + p_ * (8192 * 128) + q_ * (2048 * 128),
                   ap=[[128, 2048], [1, 128]])


def fap(base, off, dims):
    """Free-dim re-stride of an AP (partition dim must come via slicing)."""
    return bass.AP(tensor=base.tensor, offset=base.offset + off, ap=dims)


def dap(t, off, dims):
    a = t.ap()
    return bass.AP(tensor=a.tensor, offset=a.offset + off, ap=dims)


# ---------------------------------------------------------------- host prep
def _rotary_pos():
    inv = 1.0 / (10000.0 ** (np.arange(0, ROT, 2, dtype=np.float64) / ROT))
    def pos(n):
        f = np.arange(n, dtype=np.float64)[:, None] * inv[None, :]
        return np.concatenate([f, f], axis=1)                  # [n, 32]
    return pos(3), pos(M)


def _rel_bias(table):
    num_buckets, max_distance = 32, 128
    i, j = 3, 4
    rel = np.arange(j)[None, :] - np.arange(i)[:, None]
    n = np.maximum(-rel, 0)
    max_exact = num_buckets // 2
    nf = np.maximum(n, 1).astype(np.float32)
    large = max_exact + (
        np.log(nf / max_exact) / math.log(max_distance / max_exact)
        * (num_buckets - max_exact)
    ).astype(np.int32)
    large = np.minimum(large, num_buckets - 1)
    bucket = np.where(n < max_exact, n, large)
    return np.transpose(table[bucket], (2, 0, 1))   # [H, 3, 4], j0=null


def host_prep(inputs):
    f = {}
    for k, v in inputs.items():
        v = np.asarray(v)
        f[k] = v if v.dtype == np.int32 else v.astype(np.float32)
    pq, pk = _rotary_pos()
    scale = DH ** -0.5
    sgn = np.concatenate([-np.ones(16), np.ones(16)])

    d = {}
    # q rotary tables, layout [32, s*24 + i*8 + h]
    cq = np.zeros((ROT, BS * 24), np.float32)
    sq = np.zeros((ROT, BS * 24), np.float32)
    for i in range(3):
        for s_i in range(BS):
            cq[:, s_i * 24 + i * 8:s_i * 24 + i * 8 + 8] = \
                np.cos(pq[i])[:, None]
            sq[:, s_i * 24 + i * 8:s_i * 24 + i * 8 + 8] = \
                (np.sin(pq[i]) * sgn)[:, None]
    d["tq_cos"] = cq.astype(np.float16)
    d["tq_sin"] = sq.astype(np.float16)

    # sa k rotary tables, layout [32, s*3 + i]
    ck = np.zeros((ROT, T), np.float32)
    sk = np.zeros((ROT, T), np.float32)
    for i in range(3):
        ck[:, i::3] = np.cos(pq[i])[:, None]
        sk[:, i::3] = (np.sin(pq[i]) * sgn)[:, None]
    d["tksa_cos"] = ck.astype(np.float16)
    d["tksa_sin"] = sk.astype(np.float16)

    # cross-k rotary tables for the rb-blocked natural layout:
    # [128, 24, 2, ROT], col pr*8 + rt_ (pr = pair index)
    cb = np.zeros((128, 24, 2, ROT), np.float32)
    sb = np.zeros((128, 24, 2, ROT), np.float32)
    for pr in range(3):
        for rt_ in range(8):
            posb = pk[128 * (rt_ % 4):128 * (rt_ % 4) + 128]
            cb[:, pr * 8 + rt_, :, :] = np.cos(posb)[:, None, :]
            sb[:, pr * 8 + rt_, :, :] = (np.sin(posb) * sgn[None, :])[:, None, :]
    d["tkblk_cos"] = cb.astype(np.float16)
    d["tkblk_sin"] = sb.astype(np.float16)

    # self-attn bias+mask [128, 4]; rows i*8+h per 32-block, cols [t0 t1 t2 null]
    bias = _rel_bias(f["rel_bias_table"])
    bt = np.full((32, 4), NEG, np.float32)
    for i in range(3):
        for h in range(H):
            bt[i * 8 + h, 3] = bias[h, i, 0]
            for c in range(3):
                bt[i * 8 + h, c] = bias[h, i, c + 1] if c <= i else NEG
    d["bias_sa"] = np.tile(bt, (4, 1))
    d["identity"] = np.eye(128, dtype=np.float16)

    wqkv_sa = np.zeros((L, D, 640), np.float16)
    wq_ca = np.zeros((L, D, 512), np.float16)
    wo_sa = np.zeros((L, 64, H, 512), np.float16)
    wo_ca = np.zeros((L, 64, H, 512), np.float16)
    w1 = np.zeros((L, D, 2 * INNER), np.float16)
    wkv_ca = np.zeros((D, L * 128), np.float16)
    for l in range(L):
        g = f["sa_ln_in"][l][:, None]
        wqkv_sa[l, :, :512] = (g * f["sa_wq"][l]) * scale
        wqkv_sa[l, :, 512:] = g * f["sa_wkv"][l]
        wq_ca[l] = (f["ca_ln_in"][l][:, None] * f["ca_wq"][l]) * scale
        wo_sa[l] = f["sa_wo"][l].reshape(H, 64, 512).transpose(1, 0, 2)
        wo_ca[l] = f["ca_wo"][l].reshape(H, 64, 512).transpose(1, 0, 2)
        wf = f["ff_ln"][l][:, None] * f["ff_w1"][l]
        for b_i in range(4):
            w1[l, :, 1024 * b_i:1024 * b_i + 512] = \
                wf[:, 512 * b_i:512 * b_i + 512]
            w1[l, :, 1024 * b_i + 512:1024 * b_i + 1024] = \
                wf[:, 2048 + 512 * b_i:2048 + 512 * b_i + 512]
        wkv_ca[:, 128 * l:128 * l + 128] = f["ca_wkv"][l]
    d["wqkv_sa"] = wqkv_sa
    d["wq_ca"] = wq_ca
    d["wo_sa"] = wo_sa
    d["wo_ca"] = wo_ca
    d["w1"] = w1
    d["w2"] = f["ff_w2"].astype(np.float16)
    d["wkv_ca"] = wkv_ca
    d["wproj"] = (f["out_ln_g"][:, None] * f["proj_w"]).astype(np.float16)
    d["gouts"] = np.stack([f["sa_ln_out"], f["ca_ln_out"]], 1)   # [L, 2, D]
    d["kn_sa"] = f["sa_null"][:, 0, :].astype(np.float16)
    d["vn_sa"] = f["sa_null"][:, 1, :].astype(np.float16)
    d["kn_ca"] = f["ca_null"][:, 0, :].astype(np.float16)
    d["vn_ca"] = f["ca_null"][:, 1, :].astype(np.float16)
    te = f["time_emb_table"][np.asarray(inputs["timesteps"])]
    ctx16 = f["context"].astype(np.float16)
    d["_shards"] = []
    for c in range(NCORES):
        s0 = c * BS
        t0 = np.zeros((T, D), np.float32)
        t0[0::3] = te[s0:s0 + BS]
        t0[1::3] = f["x"][s0:s0 + BS]
        t0[2::3] = f["learned_query"][None, :]
        d["_shards"].append({
            "ctx": np.ascontiguousarray(ctx16[s0:s0 + BS].reshape(R, D)),
            "t0": np.ascontiguousarray(t0),
        })
    return d


# ---------------------------------------------------------------- device prog
def build_program():
    nc = bacc.Bacc("TRN2", target_bir_lowering=False, debug=False)
    dt = {}

    def din(name, shape, dty=F16):
        dt[name] = nc.dram_tensor(name, list(shape), dty, kind="ExternalInput")

    din("ctx", [R, D]); din("t0", [T, D], F32)
    din("tq_cos", [ROT, BS * 24]); din("tq_sin", [ROT, BS * 24])
    din("tksa_cos", [ROT, T]); din("tksa_sin", [ROT, T])
    din("tkblk_cos", [128, 24, 2, ROT]); din("tkblk_sin", [128, 24, 2, ROT])
    din("bias_sa", [128, 4], F32); din("identity", [128, 128])
    din("wqkv_sa", [L, D, 640]); din("wq_ca", [L, D, 512])
    din("wo_sa", [L, 64, H, 512]); din("wo_ca", [L, 64, H, 512])
    din("w1", [L, D, 2 * INNER]); din("w2", [L, INNER, D])
    din("wkv_ca", [D, L * 128]); din("wproj", [D, D])
    din("gouts", [L, 2, D], F32)
    din("kn_sa", [L, 64]); din("vn_sa", [L, 64])
    din("kn_ca", [L, 64]); din("vn_ca", [L, 64])
    out_d = nc.dram_tensor("out", [BS, D], F32, kind="ExternalOutput")

    with tile.TileContext(nc) as tc, ExitStack() as ctx:
        cst = ctx.enter_context(tc.tile_pool(name="cst", bufs=1))
        kvp = ctx.enter_context(tc.tile_pool(name="kvp", bufs=1))
        wrk = ctx.enter_context(tc.tile_pool(name="wrk", bufs=1))
        att = ctx.enter_context(tc.tile_pool(name="att", bufs=2))
        wgt = ctx.enter_context(tc.tile_pool(name="wgt", bufs=2))
        ctxp_cm = tc.tile_pool(name="ctxp", bufs=2)
        knp_cm = tc.tile_pool(name="knp", bufs=2)
        ctxp = ctxp_cm.__enter__()
        knp = knp_cm.__enter__()
        psb = ctx.enter_context(tc.tile_pool(name="psb", bufs=2, space="PSUM"))
        pss = ctx.enter_context(tc.tile_pool(name="pss", bufs=2, space="PSUM"))

        # ---- consts
        ident = cst.tile([128, 128], F16)
        nc.sync.dma_start(ident, dt["identity"][:, :])
        tqc = cst.tile([ROT, BS * 24], F16)
        tqs = cst.tile([ROT, BS * 24], F16)
        nc.sync.dma_start(tqc, dt["tq_cos"][:, :])
        nc.sync.dma_start(tqs, dt["tq_sin"][:, :])
        tkc = cst.tile([ROT, T], F16)
        tks = cst.tile([ROT, T], F16)
        nc.sync.dma_start(tkc, dt["tksa_cos"][:, :])
        nc.sync.dma_start(tks, dt["tksa_sin"][:, :])
        eps_t = cst.tile([128, 1], F32)
        nc.vector.memset(eps_t, 1e-5)
        magic = cst.tile([128, 1], mybir.dt.int32)
        nc.vector.memset(magic, 0x5f3759df)
        knull_a = cst.tile([128, L], F16)
        for half in range(2):
            nc.sync.dma_start(
                knull_a[64 * half:64 * half + 64, :],
                dt["kn_ca"][:, :].rearrange("l d -> d l"))
        vnull_a = cst.tile([128, L, 64], F16)
        nc.gpsimd.dma_start(
            out=vnull_a, in_=dap(dt["vn_ca"], 0, [[0, 128], [64, L], [1, 64]]))
        knsa_a = cst.tile([64, L], F16)
        nc.sync.dma_start(knsa_a, dt["kn_sa"][:, :].rearrange("l d -> d l"))
        bias_sa = cst.tile([128, 4], F32)
        nc.sync.dma_start(bias_sa, dt["bias_sa"][:, :])
        wkv_sb = cst.tile([128, 4, L * 128], F16)
        nc.sync.dma_start(
            wkv_sb, dt["wkv_ca"][:, :].rearrange("(c p) n -> p c n", p=128))
        kbc = knp.tile([128, 24, 2, ROT], F16, tag="kbc", bufs=1, name="kbc")
        kbs = knp.tile([128, 24, 2, ROT], F16, tag="kbs", bufs=1, name="kbs")
        nc.sync.dma_start(kbc, dt["tkblk_cos"][:, :, :, :])
        nc.sync.dma_start(kbs, dt["tkblk_sin"][:, :, :, :])

        # residual stream t [48, 512] fp32, rows s*3+i
        tres = cst.tile([T, D], F32, tag="tres", bufs=2)
        nc.sync.dma_start(tres, dt["t0"][:, :])

        # phase-0 state: kT pairs + v for all layers, staged via DRAM scratch
        kT = kvp.tile([128, 3, R], F16)         # pair p: rows 0:64 = layer 2p
        vall = kvp.tile([128, 64, L, 64], F16)  # [jc, row-tile, layer, dh]
        kscr = nc.dram_tensor("kscr", [3, R, 128], F16, kind="Internal")

        def phase0_block(rb):
            """Emit kv work for row-block rb (1024 ctx rows = 2 samples),
            all 6 layers at once; kT transposes land at odd rb."""
            cT = []
            for c in range(4):
                t_ = ctxp.tile([128, 1024], F16, tag=f"ctxT{c}",
                               name=f"ctxT{c}")
                nc.sync.dma_start_transpose(
                    t_, dt["ctx"][1024 * rb:1024 * rb + 1024,
                                  128 * c:128 * c + 128])
                cT.append(t_)
            knb = knp.tile([128, 3, 8, 128], F16, tag="knb", name="knb")
            for rt_ in range(8):
                rt = 8 * rb + rt_
                pA = psb.tile([128, 512], F32, tag="ps_big", name="pA")
                pB = pss.tile([128, 256], F32, tag="ps_q", bufs=3, name="pB")
                for c in range(4):
                    lt = cT[c][:, 128 * rt_:128 * rt_ + 128]
                    nc.tensor.matmul(pA, lt, wkv_sb[:, c, 0:512],
                                     start=(c == 0), stop=(c == 3))
                    nc.tensor.matmul(pB, lt, wkv_sb[:, c, 512:768],
                                     start=(c == 0), stop=(c == 3))
                for (ps, l0, nl) in ((pA, 0, 4), (pB, 4, 2)):
                    pv = ps.rearrange("p (l c) -> p l c", c=128)
                    ob = (l0 // 2) * 1024 + rt_ * 128
                    nc.vector.tensor_copy(
                        out=fap(knb, ob,
                                [knb.ap[0], [1024, nl // 2], [64, 2],
                                 [1, 64]]),
                        in_=fap(pv, 0, [pv.ap[0], [128, nl], [1, 64]]))
                    nc.scalar.copy(
                        out=vall[:, rt, l0:l0 + nl, :],
                        in_=fap(pv, 64, [pv.ap[0], [128, nl], [1, 64]]))
            rt1 = knp.tile([128, 24, 2, ROT], F16, tag="rt1", bufs=1,
                           name="rt1")
            rt2 = knp.tile([128, 24, 2, ROT], F16, tag="rt2", bufs=1,
                           name="rt2")
            rotv = [knb.ap[0], [128, 24], [64, 2], [1, ROT]]
            swpv = [knb.ap[0], [128, 24], [64, 2], [-16, 2], [1, 16]]
            nc.vector.tensor_tensor(out=rt1, in0=fap(knb, 0, rotv),
                                    in1=kbc, op=ALU.mult)
            nc.vector.tensor_tensor(out=rt2, in0=fap(knb, 16, swpv),
                                    in1=kbs, op=ALU.mult)
            nc.vector.tensor_tensor(out=fap(knb, 0, rotv), in0=rt1,
                                    in1=rt2, op=ALU.add)
            for p_ in range(3):
                nc.sync.dma_start(
                    dap(kscr, p_ * R * 128 + rb * 8 * 16384,
                        [[128, 128], [16384, 8], [1, 128]]),
                    knb[:, p_, :, :])
            if rb % 2 == 1:
                q_ = rb // 2
                for p_ in range(3):
                    nc.sync.dma_start_transpose(
                        kT[:, p_, 2048 * q_:2048 * q_ + 2048],
                        dt_kscr_slice(kscr, p_, q_))

        # ---- helpers
        def layernorm_fp16(src, tag):
            st = wrk.tile([T, 6], F32, tag="ln_st", name=tag + "_st")
            nc.vector.bn_stats(out=st, in_=src)
            mv = wrk.tile([T, 2], F32, tag="ln_mv", name=tag + "_mv")
            nc.vector.bn_aggr(out=mv, in_=st)
            ve = wrk.tile([T, 1], F32, tag="ln_ve", name=tag + "_ve")
            nc.vector.tensor_tensor(out=ve, in0=mv[:, 1:2], in1=eps_t[0:T, :],
                                    op=ALU.add)
            sh_i = wrk.tile([T, 1], mybir.dt.int32, tag="ln_shi",
                            name=tag + "_shi")
            nc.vector.tensor_scalar(out=sh_i, in0=ve.bitcast(mybir.dt.int32),
                                    scalar1=1, scalar2=None,
                                    op0=ALU.logical_shift_right)
            rstd = wrk.tile([T, 1], F32, tag="ln_rs", bufs=2,
                            name=tag + "_rs")
            nc.vector.tensor_tensor(out=rstd.bitcast(mybir.dt.int32),
                                    in0=magic[0:T, :], in1=sh_i,
                                    op=ALU.subtract)
            for _ in range(1):
                y2 = wrk.tile([T, 1], F32, tag="ln_y2", name=tag + "_y2")
                nc.vector.tensor_tensor(out=y2, in0=rstd, in1=rstd,
                                        op=ALU.mult)
                nc.vector.tensor_tensor(out=y2, in0=y2, in1=ve, op=ALU.mult)
                nc.vector.tensor_scalar(out=y2, in0=y2, scalar1=-0.5,
                                        scalar2=1.5, op0=ALU.mult,
                                        op1=ALU.add)
                nc.vector.tensor_tensor(out=rstd, in0=rstd, in1=y2,
                                        op=ALU.mult)
            nmu = wrk.tile([T, 1], F32, tag="ln_nm", bufs=2,
                           name=tag + "_nm")
            nc.vector.scalar_tensor_tensor(
                out=nmu, in0=mv[:, 0:1], scalar=-1.0, in1=rstd,
                op0=ALU.mult, op1=ALU.mult)
            xn = wrk.tile([T, D], F16, tag="ln_xn", bufs=2, name=tag + "_xn")
            nc.scalar.activation(out=xn, in_=src, func=AF.Identity,
                                 bias=nmu, scale=rstd)
            return xn

        def transpose_T(xn, tag):
            """[48, 512] -> [128, 4, T] via PE transpose + one ACT copy."""
            pt = pss.tile([128, 4, T], F16, tag="ps_tr", bufs=1,
                          name=tag + "_pt")
            for c in range(4):
                nc.tensor.transpose(pt[:, c, :], xn[:, 128 * c:128 * c + 128],
                                    ident[0:T, 0:T])
            xT = wrk.tile([128, 4, T], F16, tag=tag, bufs=2, name=tag)
            nc.scalar.copy(out=xT, in_=pt)
            return xT

        def q_project(xnT, w_sb, col0, tag, dup):
            """q matmuls + head-restructure + rotary.
            w_sb: [128, 4, >=col0+512]; returns qrot [rows, BS, 24] fp16."""
            qraw = wrk.tile([64, BS, 24], F16, tag="q_qraw",
                            name=tag + "_qraw")
            for mc in range(4):
                pq = pss.tile([128, T], F32, tag="ps_q", bufs=3, name="pq")
                for kc in range(4):
                    nc.tensor.matmul(
                        pq, w_sb[:, kc, col0 + 128 * mc:col0 + 128 * mc + 128],
                        xnT[:, kc, :], start=(kc == 0), stop=(kc == 3))
                for hp in range(2):
                    h = 2 * mc + hp
                    if hp == 0:
                        nc.vector.tensor_copy(
                            out=fap(qraw, h, [qraw.ap[0], [24, BS], [8, 3]]),
                            in_=pq[0:64, :])
                    else:
                        qst = wrk.tile([64, T], F32, tag="q_qst",
                                       name=tag + "_qst")
                        nc.vector.stream_shuffle(qst[0:32, :], pq[64:96, :],
                                                 IDM32)
                        nc.vector.stream_shuffle(qst[32:64, :], pq[96:128, :],
                                                 IDM32)
                        nc.vector.tensor_copy(
                            out=fap(qraw, h, [qraw.ap[0], [24, BS], [8, 3]]),
                            in_=qst)
            rows = 128 if dup else 64
            qr = wrk.tile([rows, BS, 32], F16, tag="q_qrot",
                          name=tag + "_qrot")
            nc.vector.memset(qr, 0.0)
            qf = qraw.rearrange("p a b -> p (a b)")
            qrf = qr.rearrange("p a b -> p (a b)")
            sh = wrk.tile([ROT, BS * 24], F16, tag="q_qsh",
                          name=tag + "_qsh")
            nc.vector.stream_shuffle(sh, qf[0:ROT, :], SWAP16)
            t1 = wrk.tile([ROT, BS * 24], F16, tag="q_qt1",
                          name=tag + "_qt1")
            t2 = wrk.tile([ROT, BS * 24], F16, tag="q_qt2",
                          name=tag + "_qt2")
            nc.vector.tensor_tensor(out=t1, in0=qf[0:ROT, :], in1=tqc,
                                    op=ALU.mult)
            nc.vector.tensor_tensor(out=t2, in0=sh, in1=tqs, op=ALU.mult)
            qro_r = fap(qr, 0, [[qr.ap[0][0], ROT], [32, BS], [1, 24]])
            nc.vector.tensor_tensor(out=qro_r, in0=t1, in1=t2, op=ALU.add)
            qr_hi = qr[ROT:64, :, :]
            nc.vector.tensor_copy(
                out=fap(qr_hi, 0, [qr_hi.ap[0], [32, BS], [1, 24]]),
                in_=qf[ROT:64, :])
            if dup:
                for q4 in range(2):
                    nc.vector.stream_shuffle(
                        qrf[64 + 32 * q4:96 + 32 * q4, :],
                        qrf[32 * q4:32 * (q4 + 1), :], IDM32)
            return qr

        def attn_post(ao_sb, wo_name, l, y_tag):
            aoT = wrk.tile([64, H * T], F16, tag="aoT",
                           name=y_tag + "_aoT")
            for g in range(4):
                pt = pss.tile([64, 128], F16, tag="ps_sm", name="pt")
                nc.tensor.transpose(pt, ao_sb[g], ident[:, :])
                nc.scalar.copy(
                    out=fap(aoT, 12 * g, [aoT.ap[0], [3, 4], [1, 3], [T, 8]]),
                    in_=fap(pt, 0, [pt.ap[0], [32, 4], [8, 3], [1, 8]]))
            py = psb.tile([T, 512], F32, tag="ps_big", name="py")
            for hb in range(2):
                wo = wgt.tile([64, 4, 512], F16, tag="wo", name="wo")
                nc.sync.dma_start(wo, dt[wo_name][l, :, 4 * hb:4 * hb + 4, :])
                for hh in range(4):
                    h = 4 * hb + hh
                    nc.tensor.matmul(py, aoT[:, h * T:(h + 1) * T],
                                     wo[:, hh, :], start=(h == 0),
                                     stop=(h == 7))
            return py

        def post_ln_residual(py, l, which, t_old):
            xn = layernorm_fp16(py, "po")
            go = wrk.tile([T, D], F32, tag="gout", name="gout")
            nc.gpsimd.dma_start(
                out=go, in_=dap(dt["gouts"], (l * 2 + which) * D,
                                [[0, T], [1, D]]))
            yng = wrk.tile([T, D], F32, tag="yng", name="yng")
            nc.vector.tensor_tensor(out=yng, in0=xn, in1=go, op=ALU.mult)
            t_new = cst.tile([T, D], F32, tag="tres", bufs=2, name="tres_n")
            nc.vector.tensor_tensor(out=t_new, in0=t_old, in1=yng, op=ALU.add)
            return t_new

        def self_attn(l, t_cur):
            xn = layernorm_fp16(t_cur, "sa")
            xnT = transpose_T(xn, "xnT")
            wqkv = wgt.tile([128, 4, 640], F16, tag="wq", name="wqkv")
            nc.sync.dma_start(
                wqkv, dt["wqkv_sa"][l].rearrange("(c p) n -> p c n", p=128))
            qrot = q_project(xnT, wqkv, 0, "sa", dup=False)

            pkv = pss.tile([128, T], F32, tag="ps_q", bufs=3, name="pkv")
            for kc in range(4):
                nc.tensor.matmul(pkv, wqkv[:, kc, 512:640], xnT[:, kc, :],
                                 start=(kc == 0), stop=(kc == 3))
            # kT4 [64, BS, 4], cols [t0 t1 t2 null]
            kT4 = wrk.tile([64, BS, 4], F16, tag="kT4", name="kT4")
            nc.vector.tensor_copy(
                out=fap(kT4, 3, [kT4.ap[0], [4, BS]]),
                in_=fap(knsa_a, l, [knsa_a.ap[0], [0, BS]]))
            ksh = wrk.tile([ROT, T], F32, tag="ksh", name="ksh")
            nc.vector.stream_shuffle(ksh, pkv[0:ROT, :], SWAP16)
            kt1 = wrk.tile([ROT, T], F32, tag="kt1", name="kt1")
            kt2 = wrk.tile([ROT, T], F32, tag="kt2", name="kt2")
            nc.vector.tensor_tensor(out=kt1, in0=pkv[0:ROT, :], in1=tkc,
                                    op=ALU.mult)
            nc.vector.tensor_tensor(out=kt2, in0=ksh, in1=tks, op=ALU.mult)
            nc.vector.tensor_tensor(
                out=fap(kT4, 0, [[kT4.ap[0][0], ROT], [4, BS], [1, 3]]),
                in0=kt1, in1=kt2, op=ALU.add)
            k_pass = kT4[ROT:64, :, :]
            nc.vector.tensor_copy(
                out=fap(k_pass, 0, [k_pass.ap[0], [4, BS], [1, 3]]),
                in_=pkv[ROT:64, :])
            # v natural [48, 64] via one PE transpose, then scatter to v4
            vst = wrk.tile([64, T], F32, tag="vst", name="vst")
            nc.vector.stream_shuffle(vst[0:32, :], pkv[64:96, :], IDM32)
            nc.vector.stream_shuffle(vst[32:64, :], pkv[96:128, :], IDM32)
            vst16 = wrk.tile([64, T], F16, tag="vst16", name="vst16")
            nc.vector.tensor_copy(out=vst16, in_=vst)
            v4 = wrk.tile([4, BS, 64], F16, tag="v4", name="v4")
            nc.sync.dma_start(
                v4[3:4, :, :],
                dap(dt["vn_sa"], l * 64, [[0, 1], [0, BS], [1, 64]]))
            for s in range(BS):
                pv4 = pss.tile([3, 64], F16, tag="ps_sm", name="pv4")
                nc.tensor.transpose(pv4, vst16[:, 3 * s:3 * s + 3],
                                    ident[0:64, 0:64])
                nc.vector.tensor_copy(out=v4[0:3, s, :], in_=pv4)
            # loop 1: sim + softmax for all groups (PE runs all sims first)
            at4s, rss = [], []
            for g in range(4):
                psim = pss.tile([128, 4], F32, tag="ps_sm", name="psim")
                for m_ in range(4):
                    s = 4 * g + m_
                    nc.tensor.matmul(psim[32 * m_:32 * m_ + 32, :],
                                     qrot[:, s, :], kT4[:, s, :],
                                     start=True, stop=True,
                                     tile_position=(0, 32 * m_))
                sb4 = att.tile([128, 4], F32, tag="sb4", name="sb4")
                nc.vector.tensor_tensor(out=sb4, in0=psim, in1=bias_sa,
                                        op=ALU.add)
                at4 = att.tile([128, 4], F16, tag="at4", bufs=4, name="at4")
                sums = att.tile([128, 1], F32, tag="sums4", bufs=4,
                                name="sums4")
                nc.scalar.activation(out=at4, in_=sb4, func=AF.Exp,
                                     accum_out=sums)
                rs = att.tile([128, 1], F32, tag="rs4", bufs=4, name="rs4")
                nc.vector.reciprocal(out=rs, in_=sums)
                at4s.append(at4); rss.append(rs)
            # loop 2: transpose + attn@v + scale
            ao_sb = []
            for g in range(4):
                pt4 = pss.tile([4, 128], F16, tag="ps_sm", name="pt4")
                nc.tensor.transpose(pt4, at4s[g], ident[:, :])
                atT = att.tile([4, 128], F16, tag="atT4", name="atT4")
                nc.vector.tensor_copy(out=atT, in_=pt4)
                pao = pss.tile([128, 64], F32, tag="ps_sm", name="pao")
                for m_ in range(4):
                    s = 4 * g + m_
                    nc.tensor.matmul(pao[32 * m_:32 * m_ + 32, :],
                                     atT[:, 32 * m_:32 * m_ + 32],
                                     v4[:, s, :], start=True, stop=True,
                                     tile_position=(0, 32 * m_))
                ao = att.tile([128, 64], F16, tag="ao", bufs=5, name="ao")
                nc.scalar.activation(out=ao, in_=pao, func=AF.Copy,
                                     scale=rss[g])
                ao_sb.append(ao)
            py = attn_post(ao_sb, "wo_sa", l, "ysa")
            return post_ln_residual(py, l, 0, t_cur)

        def cross_attn(l, t_cur, fill=None):
            lpair, lodd = l // 2, (l % 2) * 64
            xn = layernorm_fp16(t_cur, "ca")
            xnT = transpose_T(xn, "xnT")
            wqc = wgt.tile([128, 4, 512], F16, tag="wq", name="wqc")
            nc.sync.dma_start(
                wqc, dt["wq_ca"][l].rearrange("(c p) n -> p c n", p=128))
            qrot2 = q_project(xnT, wqc, 0, "ca", dup=True)

            knull = knull_a[:, l:l + 1]
            vnull = vnull_a[:, l, :]
            ao_sb = []
            atT = att.tile([128, 16, 128], F16, tag="atT", bufs=1, name="atT")
            sums = att.tile([128, 4], F32, tag="sums", name="sums")
            en = att.tile([128, 4], F32, tag="en", name="en")
            sums2 = att.tile([128, 4], F32, tag="sums2", name="sums2")
            an16s, rss = [], []
            for g in range(4):
                if fill is not None:
                    fill[g]()
                atn = att.tile([128, 512], F16, tag="atn", bufs=2,
                               name="atn")
                psim = psb.tile([128, 512], F32, tag="ps_big", name="psimc")
                pnull = pss.tile([128, 1], F32, tag="ps_sm", name="pnull")
                for m_ in range(4):
                    s = 4 * g + m_
                    lhs = qrot2[lodd:lodd + 64, s, :]
                    nc.tensor.matmul(
                        psim[32 * m_:32 * m_ + 32, :], lhs,
                        kT[lodd:lodd + 64, lpair, 512 * s:512 * s + 512],
                        start=True, stop=True, tile_position=(lodd, 32 * m_))
                    nc.tensor.matmul(
                        pnull[32 * m_:32 * m_ + 32, :], lhs,
                        knull[lodd:lodd + 64, :],
                        start=True, stop=True, tile_position=(lodd, 32 * m_))
                for m_ in range(4):
                    nc.vector.memset(psim[32 * m_:32 * m_ + 16, 511:512], NEG)
                    nc.vector.memset(psim[32 * m_:32 * m_ + 8, 510:511], NEG)
                nc.scalar.activation(out=atn, in_=psim, func=AF.Exp,
                                     accum_out=sums[:, g:g + 1])
                nc.scalar.activation(out=en[:, g:g + 1], in_=pnull,
                                     func=AF.Exp)
                nc.vector.tensor_tensor(out=sums2[:, g:g + 1],
                                        in0=sums[:, g:g + 1],
                                        in1=en[:, g:g + 1], op=ALU.add)
                rs_g = att.tile([128, 1], F32, tag="rs", bufs=4, name="rs_g")
                nc.vector.reciprocal(out=rs_g, in_=sums2[:, g:g + 1])
                an_g = att.tile([128, 1], F16, tag="an16", bufs=4,
                                name="an_g")
                nc.vector.tensor_copy(out=an_g, in_=en[:, g:g + 1])
                rss.append(rs_g); an16s.append(an_g)
                nc.sync.dma_start_transpose(
                    atT[:, 4 * g:4 * g + 4, :], atn)
            for g in range(4):
                pao = pss.tile([128, 64], F32, tag="ps_sm", name="paoc")
                for m_ in range(4):
                    s = 4 * g + m_
                    for jc in range(4):
                        nc.tensor.matmul(
                            pao[32 * m_:32 * m_ + 32, :],
                            atT[:, 4 * g + jc, 32 * m_:32 * m_ + 32],
                            vall[:, 4 * s + jc, l, :],
                            start=(jc == 0), stop=(jc == 3),
                            tile_position=(0, 32 * m_))
                aof = att.tile([128, 64], F32, tag="aof", name="aof")
                nc.vector.scalar_tensor_tensor(
                    out=aof, in0=vnull, scalar=an16s[g], in1=pao,
                    op0=ALU.mult, op1=ALU.add)
                ao = att.tile([128, 64], F16, tag="ao", bufs=5, name="aoc")
                nc.scalar.activation(out=ao, in_=aof, func=AF.Copy,
                                     scale=rss[g])
                ao_sb.append(ao)
            py = attn_post(ao_sb, "wo_ca", l, "yca")
            return post_ln_residual(py, l, 1, t_cur)

        late = {}

        def feed_forward(l, t_cur):
            ffp = late["p"]
            xn = layernorm_fp16(t_cur, "ff")
            xnT = transpose_T(xn, "xnT")
            ffT = ffp.tile([128, 16, T], F16, tag="ffT", name="ffT")
            pf = pss.tile([T, 512], F32, tag="ps_q", bufs=3, name="pf")
            for q4 in range(4):
                w1t = ffp.tile([128, 4, 1024], F16, tag="w1", name="w1t")
                nc.sync.dma_start(
                    w1t, dt["w1"][l, :, 1024 * q4:1024 * q4 + 1024]
                    .rearrange("(c p) n -> p c n", p=128))
                pa = psb.tile([T, 512], F32, tag="ps_big", name="pa")
                pg = psb.tile([T, 512], F32, tag="ps_big", name="pg")
                for kc in range(4):
                    nc.tensor.matmul(pa, xnT[:, kc, :], w1t[:, kc, 0:512],
                                     start=(kc == 0), stop=(kc == 3))
                for kc in range(4):
                    nc.tensor.matmul(pg, xnT[:, kc, :], w1t[:, kc, 512:1024],
                                     start=(kc == 0), stop=(kc == 3))
                # silu(g) = g * 0.5*(1 + tanh(g/2)); tanh shares the exp
                # ACT table set, so no ACT_TABLE_LOAD thrash
                th = ffp.tile([T, 512], F16, tag="th", name="th")
                nc.scalar.activation(out=th, in_=pg, func=AF.Tanh, scale=0.5)
                hs = ffp.tile([T, 512], F16, tag="hs", name="hs")
                nc.vector.tensor_scalar(out=hs, in0=th, scalar1=0.5,
                                        scalar2=0.5, op0=ALU.mult,
                                        op1=ALU.add)
                gs = ffp.tile([T, 512], F16, tag="gs", name="gs")
                nc.vector.tensor_tensor(out=gs, in0=pg, in1=hs, op=ALU.mult)
                ffin = ffp.tile([T, 512], F16, tag="ffin", bufs=2,
                                 name="ffin")
                nc.vector.tensor_tensor(out=ffin, in0=pa, in1=gs,
                                        op=ALU.mult)
                ptr = pss.tile([128, 4, T], F16, tag="ps_tr", bufs=1,
                               name="ff_pt")
                for c in range(4):
                    nc.tensor.transpose(ptr[:, c, :],
                                        ffin[:, 128 * c:128 * c + 128],
                                        ident[0:T, 0:T])
                nc.scalar.copy(out=ffT[:, 4 * q4:4 * q4 + 4, :], in_=ptr)
                w2t = ffp.tile([128, 4, 512], F16, tag="w2", name="w2t")
                nc.sync.dma_start(
                    w2t, dt["w2"][l, 512 * q4:512 * q4 + 512, :]
                    .rearrange("(c p) n -> p c n", p=128))
                for kk in range(4):
                    kc = 4 * q4 + kk
                    nc.tensor.matmul(pf, ffT[:, kc, :], w2t[:, kk, :],
                                     start=(kc == 0), stop=(kc == 15))
            t_new = cst.tile([T, D], F32, tag="tres", bufs=2, name="tres_f")
            nc.vector.tensor_tensor(out=t_new, in0=t_cur, in1=pf, op=ALU.add)
            return t_new

        # ---- schedule: SA-0 first (pure latency chain), then cross-attn
        # layer 0 with the whole phase-0 kv pipeline interleaved into its
        # group loop (group g only needs context rows of samples 4g..4g+3).
        # The kv pools are closed right after so the FF weight pool can
        # reuse their SBUF space.
        t_cur = self_attn(0, tres)

        def mkfill(g):
            def f():
                phase0_block(2 * g)
                phase0_block(2 * g + 1)
            return f

        t_cur = cross_attn(0, t_cur, fill=[mkfill(g) for g in range(4)])
        knp_cm.__exit__(None, None, None)
        ctxp_cm.__exit__(None, None, None)
        late["p"] = ctx.enter_context(tc.tile_pool(name="ffp", bufs=2))
        t_cur = feed_forward(0, t_cur)
        for l in range(1, L):
            t_cur = self_attn(l, t_cur)
            t_cur = cross_attn(l, t_cur)
            t_cur = feed_forward(l, t_cur)

        # ---- final LN + proj (last token).  The reference's "stable"
        # pre-division by max cancels exactly (LN is scale-invariant per
        # row up to the eps term), so it is skipped.
        xnf = layernorm_fp16(t_cur, "fin")
        xfT = transpose_T(xnf, "xfT")
        wp = late["p"].tile([128, 4, 512], F16, tag="wp", bufs=1, name="wp")
        nc.sync.dma_start(
            wp, dt["wproj"][:, :].rearrange("(c p) n -> p c n", p=128))
        po = psb.tile([BS, 512], F32, tag="ps_big", name="po")
        for kc in range(4):
            nc.tensor.matmul(po, fap(xfT, kc * T + 2, [xfT.ap[0], [3, BS]]),
                             wp[:, kc, :], start=(kc == 0), stop=(kc == 3))
        osb = late["p"].tile([BS, 512], F32, tag="osb", name="osb")
        nc.vector.tensor_copy(out=osb, in_=po)
        nc.sync.dma_start(out_d[:, :], osb)

    nc.compile()
    return nc


_NC_CACHE = None
LAST_RESULTS = None
_IN_NAMES = ["tq_cos", "tq_sin", "tksa_cos", "tksa_sin", "tkblk_cos",
             "tkblk_sin", "bias_sa", "identity", "wqkv_sa", "wq_ca", "wo_sa",
             "wo_ca", "w1", "w2", "wkv_ca", "wproj", "gouts", "kn_sa",
             "vn_sa", "kn_ca", "vn_ca"]


def kernel(**inputs):
    global _NC_CACHE
    d = host_prep(inputs)
    if _NC_CACHE is None:
        _NC_CACHE = build_program()
    common = {n: np.ascontiguousarray(d[n]) for n in _IN_NAMES}
    in_maps = []
    for c in range(NCORES):
        m_ = dict(common)
        m_.update(d["_shards"][c])
        in_maps.append(m_)
    import os
    trace = bool(os.environ.get("KERNEL_TRACE"))
    res = run_bass_kernel_spmd(_NC_CACHE, in_maps, core_ids=list(range(NCORES)),
                               trace=trace)
    global LAST_RESULTS
    LAST_RESULTS = res
    return np.concatenate([r["out"] for r in res.results], 0).astype(np.float32)
